# revision 3
# baseline (speedup 1.0000x reference)
"""Trainium2 Bass kernel for the CSTR (evaporator) 1M-step scan.

Strategy: parallel-in-time. The per-step map is contractive (measured
~0.965/step slow mode), so the trajectory is split into 1024 segments
(8 cores x 128 lanes) of L=1024 steps, each extended K=512 steps back
("spin-up") so an arbitrary segment-entry state converges to the true
state below fp32 noise before the graded region starts. Within each
lane's W=K+L window the nonlinear recurrence is solved by Picard-
Gauss-Seidel sweeps whose linear-recurrence cores run at line rate on
the vector engine's native tensor_tensor_scan instruction:

  x0' = x0*(SC(u0) - c02*x0 - c03*x1) + c01   (frozen-coefficient scan)
  x1' = SA(u1)*x1 + a10*x0 + SB(u0,u1)        (exact scan given x0)

4 sweeps reach the fp32 floor (rel err ~1e-6 vs the jax reference).
All param-derived scalars are passed as per-partition [128,1] operands,
so the compiled program is input-independent.
"""

import numpy as np

T = 1048576
P = 128
NCORES = 8
L = 1024          # graded steps per lane
K = 512           # spin-up steps
W = K + L         # window length per lane
TC = T // NCORES  # steps per core
SLAB = TC + K     # u rows staged per core
NSWEEPS = 4
NC_CONST = 17

# fixed model constants (match reference.py)
A, B, C_, D, E, F_, G, H = 0.5616, 0.3126, 48.43, 0.507, 55.0, 0.1538, 90.0, 0.16

_cache = {}


def _build_nc():
    if "nc" in _cache:
        return _cache["nc"]
    from contextlib import ExitStack
    import concourse.bacc as bacc
    import concourse.tile as tile
    import concourse.mybir as mybir
    from bass_rust import AP

    f32 = mybir.dt.float32
    op = mybir.AluOpType
    nc = bacc.Bacc("TRN2", target_bir_lowering=False, debug=False,
                   enable_asserts=True, num_devices=NCORES)

    uslab = nc.dram_tensor("uslab", [SLAB, 2], f32, kind="ExternalInput").ap()
    zrow = nc.dram_tensor("zrow", [1, 2 * W], f32, kind="ExternalInput").ap()
    cons = nc.dram_tensor("cons", [P, NC_CONST], f32, kind="ExternalInput").ap()
    o0 = nc.dram_tensor("o0", [P, L], f32, kind="ExternalOutput").ap()
    o1 = nc.dram_tensor("o1", [P, L], f32, kind="ExternalOutput").ap()
    oz = nc.dram_tensor("oz", [2, L], f32, kind="ExternalOutput").ap()

    # overlapped-window view of the slab: lane p reads rows [p*L, p*L + W)
    win = AP(uslab.tensor, 0, [[L * 2, P], [1, 2 * W]])

    with tile.TileContext(nc) as tc, ExitStack() as ctx:
        pool = ctx.enter_context(tc.tile_pool(name="main", bufs=1))
        t_uw = pool.tile([P, 2 * W], f32)
        t_cons = pool.tile([P, NC_CONST], f32)
        nc.sync.dma_start(t_uw[:], win)
        nc.sync.dma_start(t_uw[0:1, :], zrow[0:1, :])
        nc.sync.dma_start(t_cons[:], cons[:])

        def cst(i):
            return t_cons[:, i : i + 1]

        t_u0 = pool.tile([P, W], f32)
        t_u1 = pool.tile([P, W], f32)
        t_d = pool.tile([P, W], f32)
        t_scr = pool.tile([P, W], f32)
        t_rec = pool.tile([P, W], f32)
        t_r = pool.tile([P, W], f32)
        t_SA = pool.tile([P, W], f32)
        t_SBp = pool.tile([P, W], f32)
        t_SB = pool.tile([P, W], f32)
        t_SC = pool.tile([P, W], f32)
        t_b = pool.tile([P, W], f32)
        t_v = pool.tile([P, W - 1], f32)
        t_a = pool.tile([P, W - 1], f32)
        t_c = pool.tile([P, W - 1], f32)
        t_X0 = pool.tile([P, W], f32)
        t_X1 = pool.tile([P, W], f32)

        ident = mybir.ActivationFunctionType.Identity

        # deinterleave u0/u1 from pair-interleaved windows
        nc.vector.tensor_copy(t_u0[:], t_uw[:, 0::2])
        nc.scalar.copy(t_u1[:], t_uw[:, 1::2])

        # r(u1) = (2*Cp*UA2*u1) / (2*Cp*u1 + UA2)
        nc.scalar.activation(t_d[:], t_u1[:], ident, bias=cst(1), scale=cst(0))
        nc.vector.reciprocal_approx_accurate(t_rec[:], t_d[:], t_scr[:])
        nc.vector.scalar_tensor_tensor(t_r[:], t_u1[:], cst(2), t_rec[:],
                                       op.mult, op.mult)
        # SA = cA1 + cA2*r ; SB = (cB1 + cB2*u0) + cB3*r ; SC = cC1 + cC2*u0
        nc.vector.tensor_scalar(t_SA[:], t_r[:], cst(3), cst(4), op.mult, op.add)
        nc.scalar.activation(t_SBp[:], t_u0[:], ident, bias=cst(6), scale=cst(5))
        nc.vector.scalar_tensor_tensor(t_SB[:], t_r[:], cst(7), t_SBp[:],
                                       op.mult, op.add)
        nc.scalar.activation(t_SC[:], t_u0[:], ident, bias=cst(9), scale=cst(8))
        # b = const c01 tile
        nc.vector.tensor_scalar(t_b[:], t_u0[:], 0.0, cst(11), op.mult, op.add)

        # init states (column 0 never changes across sweeps)
        nc.vector.tensor_copy(t_X0[:, 0:1], cst(15))
        nc.vector.tensor_copy(t_X1[:, 0:1], cst(16))

        Wm = W - 1
        for s in range(NSWEEPS):
            if s == 0:
                # X0bar/X1bar = const init: a = SC - (c02*i0 + c03*i1)
                nc.vector.tensor_scalar(t_a[:], t_SC[:, 0:Wm], cst(10), None,
                                        op.subtract)
            else:
                nc.vector.scalar_tensor_tensor(t_v[:], t_X0[:, 0:Wm], cst(13),
                                               t_SC[:, 0:Wm], op.mult, op.add)
                nc.vector.scalar_tensor_tensor(t_a[:], t_X1[:, 0:Wm], cst(14),
                                               t_v[:], op.mult, op.add)
            nc.vector.tensor_tensor_scan(t_X0[:, 1:W], t_a[:], t_b[:, 0:Wm],
                                         t_X0[:, 0:1], op.mult, op.add)
            nc.vector.scalar_tensor_tensor(t_c[:], t_X0[:, 0:Wm], cst(12),
                                           t_SB[:, 0:Wm], op.mult, op.add)
            nc.vector.tensor_tensor_scan(t_X1[:, 1:W], t_SA[:, 0:Wm], t_c[:],
                                         t_X1[:, 0:1], op.mult, op.add)

        nc.sync.dma_start(o0[:], t_X0[:, K : K + L])
        nc.sync.dma_start(o1[:], t_X1[:, K : K + L])
        nc.sync.dma_start(oz[0:1, :], t_X0[0:1, 0:L])
        nc.sync.dma_start(oz[1:2, :], t_X1[0:1, 0:L])

    nc.compile()
    _cache["nc"] = nc
    return nc


def _make_in_maps(u, x0, params):
    u = np.ascontiguousarray(u, np.float32)
    M, Cc, UA2, Cp, lam, lams, F1, X1p, F3, T1, T200 = [float(params[i]) for i in range(11)]
    UA1 = H * (F1 + F3)
    k1 = (UA1 + F1 * Cp) / lam
    p_ = k1 * B
    q_ = k1 * A
    alpha_u = UA1 * F_ / lam
    alpha_c = (UA1 * G + F1 * Cp * T1) / lam - k1 * C_
    c01 = F1 * X1p / M
    c02 = p_ / M
    c03 = q_ / M
    a10 = -p_ / Cc
    i0, i1 = float(x0[0]), float(x0[1])

    cvals = np.zeros(NC_CONST, np.float64)
    cvals[0] = 2.0 * Cp                     # scale_d
    cvals[1] = UA2                          # bias_d
    cvals[2] = 2.0 * Cp * UA2               # rmul
    cvals[3] = -D / (lam * Cc)              # cA2
    cvals[4] = 1.0 - q_ / Cc                # cA1
    cvals[5] = alpha_u / Cc                 # cB2
    cvals[6] = alpha_c / Cc                 # cB1
    cvals[7] = -(E - T200) / (lam * Cc)     # cB3
    cvals[8] = alpha_u / M                  # cC2
    cvals[9] = 1.0 - (F1 - alpha_c) / M     # cC1
    cvals[10] = c02 * i0 + c03 * i1         # sw1
    cvals[11] = c01
    cvals[12] = a10
    cvals[13] = -c02
    cvals[14] = -c03
    cvals[15] = i0
    cvals[16] = i1
    cons = np.tile(cvals.astype(np.float32)[None, :], (P, 1))

    in_maps = []
    for c in range(NCORES):
        if c == 0:
            slab = np.concatenate([np.repeat(u[0:1], K, axis=0), u[0:TC]], axis=0)
            zr = u[0:W].reshape(1, 2 * W)
        else:
            slab = u[c * TC - K : c * TC + TC]
            zr = slab[0:W].reshape(1, 2 * W)
        in_maps.append({
            "uslab": np.ascontiguousarray(slab),
            "zrow": np.ascontiguousarray(zr),
            "cons": cons,
        })
    return in_maps


def _host_head(u, x0, params, n):
    # exact fp32 simulation of the first n steps (lane 0 has no spin-up
    # protection, so its early positions converge slowly on-device; the
    # first K steps are 0.05% of the work and are computed here instead)
    f = np.float32
    M, Cc, UA2, Cp, lam, lams, F1, X1p, F3, T1, T200 = [f(params[i]) for i in range(11)]
    out = np.empty((n, 2), f)
    s0, s1 = f(x0[0]), f(x0[1])
    fA, fB, fC, fD, fE, fF, fG, fH = f(A), f(B), f(C_), f(D), f(E), f(F_), f(G), f(H)
    one, two = f(1.0), f(2.0)
    UA1 = fH * (F1 + F3)
    for t in range(n):
        out[t, 0] = s0
        out[t, 1] = s1
        u0, u1 = f(u[t, 0]), f(u[t, 1])
        T2 = fA * s1 + fB * s0 + fC
        T3 = fD * s1 + fE
        T100 = fF * u0 + fG
        Q100 = UA1 * (T100 - T2)
        Q200 = UA2 * (T3 - T200) / (one + UA2 / (two * Cp * u1))
        F5 = Q200 / lam
        F4 = (Q100 - F1 * Cp * (T2 - T1)) / lam
        F2 = F1 - F4
        X2d = (F1 * X1p - F2 * s0) / M
        P2d = (F4 - F5) / Cc
        s0 = s0 + X2d
        s1 = s1 + P2d
    return out


def _assemble(results, head):
    out = np.empty((T, 2), np.float32)
    for c in range(NCORES):
        out[c * TC : (c + 1) * TC, 0] = results[c]["o0"].reshape(-1)
        out[c * TC : (c + 1) * TC, 1] = results[c]["o1"].reshape(-1)
    out[K:L, 0] = results[0]["oz"][0, K:L]
    out[K:L, 1] = results[0]["oz"][1, K:L]
    out[0:K] = head
    return out


def run(u_forced, x0, params, trace=False):
    from concourse.bass_utils import run_bass_kernel_spmd
    nc = _build_nc()
    in_maps = _make_in_maps(u_forced, x0, params)
    head = _host_head(u_forced, x0, params, K)
    res = run_bass_kernel_spmd(nc, in_maps, list(range(NCORES)), trace=trace)
    return _assemble(res.results, head), res


def kernel(u_forced, x0, params):
    out, _ = run(u_forced, x0, params, trace=False)
    return out


# revision 8
# speedup vs baseline: 1.0509x; 1.0509x over previous
"""Trainium2 Bass kernel for the CSTR (evaporator) 1M-step scan.

Parallel-in-time: the per-step map is contractive (~0.965/step slow mode),
so the trajectory is split into 1024 segments (8 cores x 128 lanes) of
L=1024 steps, each extended K steps back ("spin-up") so an arbitrary
segment-entry state converges below fp32 noise before the graded region.
Within each lane's window the nonlinear recurrence is solved by Picard-
Gauss-Seidel sweeps whose linear-recurrence cores run on the vector
engine's native tensor_tensor_scan. States are pre-scaled (z=c02*x0,
y1=c03*x1) so the sweep coefficient ops are plain tensor_tensor ops that
GpSimd can execute, overlapped with the scans via column chunking.
Later sweeps start at column 128/256: contraction washes the suffix they
inherit, saving scan length. The first K outputs are computed on host
(0.04% of the work) since segment 0 has no spin-up protection.
All param-derived scalars are per-partition [128,1] operands, so the
compiled program is input-independent.
"""

import numpy as np

T = 1048576
P = 128
NCORES = 8
L = 1024          # graded steps per lane
K = 384           # spin-up steps
W = K + L         # window length per lane (1408)
TC = T // NCORES  # steps per core
SLAB = TC + K     # u rows staged per core
NSWEEPS = 4
SWEEP_J0 = [0, 0, 128, 256]  # start column per sweep (suffix protected by contraction)
NC_CONST = 19

# fixed model constants (match reference.py)
A, B, C_, D, E, F_, G, H = 0.5616, 0.3126, 48.43, 0.507, 55.0, 0.1538, 90.0, 0.16

_cache = {}


def _build_nc():
    if "nc" in _cache:
        return _cache["nc"]
    from contextlib import ExitStack
    import concourse.bacc as bacc
    import concourse.tile as tile
    import concourse.mybir as mybir
    from bass_rust import AP

    f32 = mybir.dt.float32
    op = mybir.AluOpType
    ident = mybir.ActivationFunctionType.Identity
    nc = bacc.Bacc("TRN2", target_bir_lowering=False, debug=False,
                   enable_asserts=True, num_devices=NCORES)

    uslab = nc.dram_tensor("uslab", [SLAB, 2], f32, kind="ExternalInput").ap()
    zrow = nc.dram_tensor("zrow", [1, 2 * W], f32, kind="ExternalInput").ap()
    cons = nc.dram_tensor("cons", [P, NC_CONST], f32, kind="ExternalInput").ap()
    o0 = nc.dram_tensor("o0", [P, L], f32, kind="ExternalOutput").ap()
    o1 = nc.dram_tensor("o1", [P, L], f32, kind="ExternalOutput").ap()
    oz = nc.dram_tensor("oz", [2, L - K], f32, kind="ExternalOutput").ap()

    Wm = W - 1
    Wh = W // 2  # precompute half width (pairs)

    with tile.TileContext(nc) as tc, ExitStack() as ctx:
        pool = ctx.enter_context(tc.tile_pool(name="main", bufs=1))
        t_uw = pool.tile([P, 2 * W], f32, tag="uw")
        t_cons = pool.tile([P, NC_CONST], f32, tag="cons")

        def cst(i):
            return t_cons[:, i : i + 1]

        t_num = pool.tile([P, W], f32, tag="num")   # recip scratch
        t_rec = pool.tile([P, W], f32, tag="rec")
        t_den = pool.tile([P, W], f32, tag="den")
        t_r = pool.tile([P, W], f32, tag="r")
        t_SA = pool.tile([P, W], f32, tag="SA")
        t_SBp = pool.tile([P, W], f32, tag="SBp")
        t_SB3 = pool.tile([P, W], f32, tag="SB3")
        t_SC = pool.tile([P, W], f32, tag="SC")
        t_b = pool.tile([P, W], f32, tag="b")
        t_t1 = pool.tile([P, Wm], f32, tag="t1")
        t_a = pool.tile([P, Wm], f32, tag="a")
        t_c = pool.tile([P, Wm], f32, tag="c")
        t_Z = pool.tile([P, W], f32, tag="Z")
        t_Y = pool.tile([P, W], f32, tag="Y")
        t_o0 = pool.tile([P, L], f32, tag="o0")
        t_o1 = pool.tile([P, L], f32, tag="o1")
        t_oz0 = pool.tile([1, L - K], f32, tag="ozz0")
        t_oz1 = pool.tile([1, L - K], f32, tag="ozz1")

        nc.sync.dma_start(t_cons[:], cons[:])
        # window DMA in halves so stream precompute overlaps the transfer
        win0 = AP(uslab.tensor, 0, [[L * 2, P], [1, W]])
        win1 = AP(uslab.tensor, W, [[L * 2, P], [1, W]])
        nc.sync.dma_start(t_uw[:, 0:W], win0)
        nc.sync.dma_start(t_uw[0:1, 0:W], zrow[0:1, 0:W])
        nc.sync.dma_start(t_uw[:, W : 2 * W], win1)
        nc.sync.dma_start(t_uw[0:1, W : 2 * W], zrow[0:1, W : 2 * W])

        # stream precompute per half; r(u1) via DVE approx reciprocal (~2 ULP)
        for h in (0, 1):
            lo, hi = h * Wh, (h + 1) * Wh
            u0h = t_uw[:, 2 * lo : 2 * hi : 2]
            u1h = t_uw[:, 2 * lo + 1 : 2 * hi : 2]
            nc.vector.tensor_scalar(t_den[:, lo:hi], u1h, cst(0), cst(1), op.mult, op.add)
            nc.vector.reciprocal_approx_accurate(t_rec[:, lo:hi], t_den[:, lo:hi],
                                                 t_num[:, lo:hi])
            nc.vector.scalar_tensor_tensor(t_r[:, lo:hi], u1h, cst(2),
                                           t_rec[:, lo:hi], op.mult, op.mult)
            nc.scalar.activation(t_SA[:, lo:hi], t_r[:, lo:hi], ident,
                                 bias=cst(4), scale=cst(3))
            nc.scalar.activation(t_SBp[:, lo:hi], u0h, ident, bias=cst(6), scale=cst(5))
            nc.vector.scalar_tensor_tensor(t_SB3[:, lo:hi], t_r[:, lo:hi], cst(7),
                                           t_SBp[:, lo:hi], op.mult, op.add)
            nc.scalar.activation(t_SC[:, lo:hi], u0h, ident, bias=cst(9), scale=cst(8))
            nc.scalar.activation(t_b[:, lo:hi], t_den[:, lo:hi], ident,
                                 bias=cst(11), scale=0.0)

        nc.vector.tensor_copy(t_Z[:, 0:1], cst(15))
        nc.vector.tensor_copy(t_Y[:, 0:1], cst(16))

        for s in range(NSWEEPS):
            j0 = SWEEP_J0[s] if s < len(SWEEP_J0) else SWEEP_J0[-1]
            mid = j0 + (Wm - j0) // 2
            bounds = [(j0, mid), (mid, Wm)]
            if s == 0:
                for lo, hi in bounds:
                    nc.vector.tensor_scalar(t_a[:, lo:hi], t_SC[:, lo:hi], cst(10),
                                            None, op.subtract)
            else:
                for lo, hi in bounds:
                    nc.gpsimd.tensor_tensor(t_t1[:, lo:hi], t_Z[:, lo:hi],
                                            t_Y[:, lo:hi], op.add)
                    nc.gpsimd.tensor_tensor(t_a[:, lo:hi], t_SC[:, lo:hi],
                                            t_t1[:, lo:hi], op.subtract)
            (l0, h0), (l1, h1) = bounds
            nc.vector.tensor_tensor_scan(t_Z[:, l0 + 1 : h0 + 1], t_a[:, l0:h0],
                                         t_b[:, l0:h0], t_Z[:, l0 : l0 + 1],
                                         op.mult, op.add)
            nc.vector.scalar_tensor_tensor(t_c[:, l0:h0], t_Z[:, l0:h0], cst(12),
                                           t_SB3[:, l0:h0], op.mult, op.add)
            nc.vector.tensor_tensor_scan(t_Z[:, l1 + 1 : h1 + 1], t_a[:, l1:h1],
                                         t_b[:, l1:h1], t_Z[:, l1 : l1 + 1],
                                         op.mult, op.add)
            nc.vector.tensor_tensor_scan(t_Y[:, l0 + 1 : h0 + 1], t_SA[:, l0:h0],
                                         t_c[:, l0:h0], t_Y[:, l0 : l0 + 1],
                                         op.mult, op.add)
            nc.vector.scalar_tensor_tensor(t_c[:, l1:h1], t_Z[:, l1:h1], cst(12),
                                           t_SB3[:, l1:h1], op.mult, op.add)
            nc.vector.tensor_tensor_scan(t_Y[:, l1 + 1 : h1 + 1], t_SA[:, l1:h1],
                                         t_c[:, l1:h1], t_Y[:, l1 : l1 + 1],
                                         op.mult, op.add)

        # tail: unscale (ACT) + DMA out, split at the final sweep's chunk edge
        jmid = SWEEP_J0[-1] + (Wm - SWEEP_J0[-1]) // 2 + 1  # first col of chunk B's output
        for lo, hi in ((K, jmid), (jmid, W)):
            nc.scalar.activation(t_o0[:, lo - K : hi - K], t_Z[:, lo:hi], ident,
                                 scale=cst(17))
            nc.sync.dma_start(o0[:, lo - K : hi - K], t_o0[:, lo - K : hi - K])
            nc.scalar.activation(t_o1[:, lo - K : hi - K], t_Y[:, lo:hi], ident,
                                 scale=cst(18))
            nc.sync.dma_start(o1[:, lo - K : hi - K], t_o1[:, lo - K : hi - K])
        nc.scalar.activation(t_oz0[:], t_Z[0:1, K:L], ident, scale=cst(17)[0:1])
        nc.scalar.activation(t_oz1[:], t_Y[0:1, K:L], ident, scale=cst(18)[0:1])
        nc.sync.dma_start(oz[0:1, :], t_oz0[:])
        nc.sync.dma_start(oz[1:2, :], t_oz1[:])

    nc.compile()
    _cache["nc"] = nc
    return nc


def _derive(params, x0):
    M, Cc, UA2, Cp, lam, lams, F1, X1p, F3, T1, T200 = [float(params[i]) for i in range(11)]
    UA1 = H * (F1 + F3)
    k1 = (UA1 + F1 * Cp) / lam
    p_ = k1 * B
    q_ = k1 * A
    alpha_u = UA1 * F_ / lam
    alpha_c = (UA1 * G + F1 * Cp * T1) / lam - k1 * C_
    c01 = F1 * X1p / M
    c02 = p_ / M
    c03 = q_ / M
    a10 = -p_ / Cc
    i0, i1 = float(x0[0]), float(x0[1])

    cv = np.zeros(NC_CONST, np.float64)
    cv[0] = 2.0 * Cp
    cv[1] = UA2
    cv[2] = 2.0 * Cp * UA2
    cv[3] = -D / (lam * Cc)                  # cA2
    cv[4] = 1.0 - q_ / Cc                    # cA1
    cv[5] = c03 * alpha_u / Cc               # cB2' (SB pre-scaled by c03)
    cv[6] = c03 * alpha_c / Cc               # cB1'
    cv[7] = c03 * (-(E - T200) / (lam * Cc)) # cB3'
    cv[8] = alpha_u / M                      # cC2
    cv[9] = 1.0 - (F1 - alpha_c) / M         # cC1
    cv[10] = c02 * i0 + c03 * i1             # sweep-1 a offset (scaled states)
    cv[11] = c02 * c01                       # scan0 additive const (scaled)
    cv[12] = c03 * a10 / c02                 # c coefficient on z
    cv[15] = c02 * i0                        # z init
    cv[16] = c03 * i1                        # y1 init
    cv[17] = 1.0 / c02                       # unscale z
    cv[18] = 1.0 / c03                       # unscale y1
    return cv.astype(np.float32)


def _make_in_maps(u, x0, params):
    u = np.ascontiguousarray(u, np.float32)
    cons = np.tile(_derive(params, x0)[None, :], (P, 1))
    in_maps = []
    for c in range(NCORES):
        if c == 0:
            slab = np.concatenate([np.repeat(u[0:1], K, axis=0), u[0:TC]], axis=0)
            zr = u[0:W].reshape(1, 2 * W)
        else:
            slab = u[c * TC - K : c * TC + TC]
            zr = slab[0:W].reshape(1, 2 * W)
        in_maps.append({
            "uslab": np.ascontiguousarray(slab),
            "zrow": np.ascontiguousarray(zr),
            "cons": cons,
        })
    return in_maps


def _host_head(u, x0, params, n):
    # exact fp32 simulation of the first n steps (segment 0 has no spin-up)
    f = np.float32
    M, Cc, UA2, Cp, lam, lams, F1, X1p, F3, T1, T200 = [f(params[i]) for i in range(11)]
    out = np.empty((n, 2), f)
    s0, s1 = f(x0[0]), f(x0[1])
    fA, fB, fC, fD, fE, fF, fG, fH = f(A), f(B), f(C_), f(D), f(E), f(F_), f(G), f(H)
    one, two = f(1.0), f(2.0)
    UA1 = fH * (F1 + F3)
    for t in range(n):
        out[t, 0] = s0
        out[t, 1] = s1
        u0, u1 = f(u[t, 0]), f(u[t, 1])
        T2 = fA * s1 + fB * s0 + fC
        T3 = fD * s1 + fE
        T100 = fF * u0 + fG
        Q100 = UA1 * (T100 - T2)
        Q200 = UA2 * (T3 - T200) / (one + UA2 / (two * Cp * u1))
        F5 = Q200 / lam
        F4 = (Q100 - F1 * Cp * (T2 - T1)) / lam
        F2 = F1 - F4
        X2d = (F1 * X1p - F2 * s0) / M
        P2d = (F4 - F5) / Cc
        s0 = s0 + X2d
        s1 = s1 + P2d
    return out


def _assemble(results, head):
    out = np.empty((T, 2), np.float32)
    for c in range(NCORES):
        out[c * TC : (c + 1) * TC, 0] = results[c]["o0"].reshape(-1)
        out[c * TC : (c + 1) * TC, 1] = results[c]["o1"].reshape(-1)
    out[K:L, 0] = results[0]["oz"][0]
    out[K:L, 1] = results[0]["oz"][1]
    out[0:K] = head
    return out


def run(u_forced, x0, params, trace=False):
    from concourse.bass_utils import run_bass_kernel_spmd
    nc = _build_nc()
    in_maps = _make_in_maps(u_forced, x0, params)
    head = _host_head(u_forced, x0, params, K)
    res = run_bass_kernel_spmd(nc, in_maps, list(range(NCORES)), trace=trace)
    return _assemble(res.results, head), res


def kernel(u_forced, x0, params):
    out, _ = run(u_forced, x0, params, trace=False)
    return out


# revision 12
# speedup vs baseline: 1.1677x; 1.1111x over previous
"""Trainium2 Bass kernel for the CSTR (evaporator) 1M-step scan.

Parallel-in-time: the per-step map is contractive (~0.965/step slow mode),
so the trajectory is split into 1024 segments (8 cores x 128 lanes) of
L=1024 steps, each extended K=384 steps back ("spin-up") so an arbitrary
segment-entry state converges below fp32 noise before the graded region.
Within each lane's window the nonlinear recurrence

  x0' = x0*(SC(u0) - c02*x0 - c03*x1) + c01
  x1' = SA(u1)*x1 + a10*x0 + SB(u0,u1)

is solved by 4 Picard-Gauss-Seidel sweeps whose linear-recurrence cores
run on the vector engine's native tensor_tensor_scan. Later sweeps start
at column 128/256: contraction washes the inherited suffix. Input DMA is
split into 4 column chunks on two DGE queues with stream precompute and
the first sweep chasing the chunks. The first K outputs are computed on
host (0.04% of the work) since segment 0 has no spin-up protection.
All param-derived scalars are per-partition [128,1] operands, so the
compiled program is input-independent.
"""

import numpy as np

T = 1048576
P = 128
NCORES = 8
L = 1024          # graded steps per lane
K = 384           # spin-up steps
W = K + L         # window length per lane (1408)
TC = T // NCORES  # steps per core
SLAB = TC + K     # u rows staged per core
NSWEEPS = 4
SWEEP_J0 = [0, 0, 128, 256]
NQ = 4            # head chunks
NC_CONST = 17

# fixed model constants (match reference.py)
A, B, C_, D, E, F_, G, H = 0.5616, 0.3126, 48.43, 0.507, 55.0, 0.1538, 90.0, 0.16

_cache = {}


def _build_nc():
    if "nc" in _cache:
        return _cache["nc"]
    from contextlib import ExitStack
    import concourse.bacc as bacc
    import concourse.tile as tile
    import concourse.mybir as mybir
    from bass_rust import AP

    f32 = mybir.dt.float32
    op = mybir.AluOpType
    ident = mybir.ActivationFunctionType.Identity
    nc = bacc.Bacc("TRN2", target_bir_lowering=False, debug=False,
                   enable_asserts=True, num_devices=NCORES)

    uslab = nc.dram_tensor("uslab", [SLAB, 2], f32, kind="ExternalInput").ap()
    zrow = nc.dram_tensor("zrow", [1, 2 * W], f32, kind="ExternalInput").ap()
    cons = nc.dram_tensor("cons", [P, NC_CONST], f32, kind="ExternalInput").ap()
    o0 = nc.dram_tensor("o0", [P, L], f32, kind="ExternalOutput").ap()
    o1 = nc.dram_tensor("o1", [P, L], f32, kind="ExternalOutput").ap()
    oz = nc.dram_tensor("oz", [2, L - K], f32, kind="ExternalOutput").ap()

    Wm = W - 1
    Wq = W // NQ

    with tile.TileContext(nc) as tc, ExitStack() as ctx:
        pool = ctx.enter_context(tc.tile_pool(name="main", bufs=1))
        t_uw = pool.tile([P, 2 * W], f32, tag="uw")
        t_cons = pool.tile([P, NC_CONST], f32, tag="cons")

        def cst(i):
            return t_cons[:, i : i + 1]

        t_scr = pool.tile([P, W], f32, tag="scr")   # recip scratch
        t_rec = pool.tile([P, W], f32, tag="rec")
        t_den = pool.tile([P, W], f32, tag="den")
        t_r = pool.tile([P, W], f32, tag="r")
        t_SA = pool.tile([P, W], f32, tag="SA")
        t_SBp = pool.tile([P, W], f32, tag="SBp")
        t_SB = pool.tile([P, W], f32, tag="SB")
        t_SC = pool.tile([P, W], f32, tag="SC")
        t_b = pool.tile([P, W], f32, tag="b")
        t_v = pool.tile([P, Wm], f32, tag="v")
        t_a = pool.tile([P, Wm], f32, tag="a")
        t_c = pool.tile([P, Wm], f32, tag="c")
        t_X0 = pool.tile([P, W], f32, tag="X0")
        t_X1 = pool.tile([P, W], f32, tag="X1")

        nc.sync.dma_start(t_cons[:], cons[:])
        # ACT table warm-up (Identity) while DMA streams in
        nc.scalar.activation(t_scr[:, 0:1], t_cons[:, 0:1], ident, bias=0.0, scale=1.0)

        # input windows: 4 column chunks alternating two DGE queues
        for q in range(NQ):
            lo = q * Wq
            eng = nc.sync if q % 2 == 0 else nc.scalar
            winq = AP(uslab.tensor, 2 * lo, [[L * 2, P], [1, 2 * Wq]])
            eng.dma_start(t_uw[:, 2 * lo : 2 * (lo + Wq)], winq)
            eng.dma_start(t_uw[0:1, 2 * lo : 2 * (lo + Wq)],
                          zrow[0:1, 2 * lo : 2 * (lo + Wq)])

        nc.vector.tensor_copy(t_X0[:, 0:1], cst(15))
        nc.vector.tensor_copy(t_X1[:, 0:1], cst(16))

        # stream precompute chases the DMA chunks
        for q in range(NQ):
            lo, hi = q * Wq, (q + 1) * Wq
            u0q = t_uw[:, 2 * lo : 2 * hi : 2]
            u1q = t_uw[:, 2 * lo + 1 : 2 * hi : 2]
            nc.vector.tensor_scalar(t_den[:, lo:hi], u1q, cst(0), cst(1), op.mult, op.add)
            nc.vector.reciprocal_approx_accurate(t_rec[:, lo:hi], t_den[:, lo:hi],
                                                 t_scr[:, lo:hi])
            nc.vector.scalar_tensor_tensor(t_r[:, lo:hi], u1q, cst(2),
                                           t_rec[:, lo:hi], op.mult, op.mult)
            nc.scalar.activation(t_SC[:, lo:hi], u0q, ident, bias=cst(9), scale=cst(8))
            nc.scalar.activation(t_SBp[:, lo:hi], u0q, ident, bias=cst(6), scale=cst(5))
            nc.scalar.activation(t_SA[:, lo:hi], t_r[:, lo:hi], ident,
                                 bias=cst(4), scale=cst(3))
            nc.scalar.activation(t_b[:, lo:hi], t_den[:, lo:hi], ident,
                                 bias=cst(11), scale=0.0)
            nc.vector.scalar_tensor_tensor(t_SB[:, lo:hi], t_r[:, lo:hi], cst(7),
                                           t_SBp[:, lo:hi], op.mult, op.add)

        # sweep 1: chunked x4, chasing precompute
        for q in range(NQ):
            lo = q * Wq
            hi = min((q + 1) * Wq, Wm)
            nc.vector.tensor_scalar(t_a[:, lo:hi], t_SC[:, lo:hi], cst(10),
                                    None, op.subtract)
            nc.vector.tensor_tensor_scan(t_X0[:, lo + 1 : hi + 1], t_a[:, lo:hi],
                                         t_b[:, lo:hi], t_X0[:, lo : lo + 1],
                                         op.mult, op.add)
        for q in range(NQ):
            lo = q * Wq
            hi = min((q + 1) * Wq, Wm)
            nc.vector.scalar_tensor_tensor(t_c[:, lo:hi], t_X0[:, lo:hi], cst(12),
                                           t_SB[:, lo:hi], op.mult, op.add)
            nc.vector.tensor_tensor_scan(t_X1[:, lo + 1 : hi + 1], t_SA[:, lo:hi],
                                         t_c[:, lo:hi], t_X1[:, lo : lo + 1],
                                         op.mult, op.add)

        # sweeps 2..N-1: full-range single ops
        for s in range(1, NSWEEPS - 1):
            j0 = SWEEP_J0[s]
            nc.vector.scalar_tensor_tensor(t_v[:, j0:Wm], t_X0[:, j0:Wm], cst(13),
                                           t_SC[:, j0:Wm], op.mult, op.add)
            nc.vector.scalar_tensor_tensor(t_a[:, j0:Wm], t_X1[:, j0:Wm], cst(14),
                                           t_v[:, j0:Wm], op.mult, op.add)
            nc.vector.tensor_tensor_scan(t_X0[:, j0 + 1 : W], t_a[:, j0:Wm],
                                         t_b[:, j0:Wm], t_X0[:, j0 : j0 + 1],
                                         op.mult, op.add)
            nc.vector.scalar_tensor_tensor(t_c[:, j0:Wm], t_X0[:, j0:Wm], cst(12),
                                           t_SB[:, j0:Wm], op.mult, op.add)
            nc.vector.tensor_tensor_scan(t_X1[:, j0 + 1 : W], t_SA[:, j0:Wm],
                                         t_c[:, j0:Wm], t_X1[:, j0 : j0 + 1],
                                         op.mult, op.add)

        # final sweep: chunked x2, output DMA inline
        j0 = SWEEP_J0[NSWEEPS - 1]
        mid = j0 + (Wm - j0) // 2
        bounds = [(j0, mid), (mid, Wm)]
        nc.vector.scalar_tensor_tensor(t_v[:, j0:Wm], t_X0[:, j0:Wm], cst(13),
                                       t_SC[:, j0:Wm], op.mult, op.add)
        nc.vector.scalar_tensor_tensor(t_a[:, j0:Wm], t_X1[:, j0:Wm], cst(14),
                                       t_v[:, j0:Wm], op.mult, op.add)
        (l0, h0), (l1, h1) = bounds
        nc.vector.tensor_tensor_scan(t_X0[:, l0 + 1 : h0 + 1], t_a[:, l0:h0],
                                     t_b[:, l0:h0], t_X0[:, l0 : l0 + 1],
                                     op.mult, op.add)
        nc.vector.scalar_tensor_tensor(t_c[:, l0:h0], t_X0[:, l0:h0], cst(12),
                                       t_SB[:, l0:h0], op.mult, op.add)
        nc.vector.tensor_tensor_scan(t_X0[:, l1 + 1 : h1 + 1], t_a[:, l1:h1],
                                     t_b[:, l1:h1], t_X0[:, l1 : l1 + 1],
                                     op.mult, op.add)
        nc.sync.dma_start(o0[:, 0 : h0 + 1 - K], t_X0[:, K : h0 + 1])
        nc.vector.tensor_tensor_scan(t_X1[:, l0 + 1 : h0 + 1], t_SA[:, l0:h0],
                                     t_c[:, l0:h0], t_X1[:, l0 : l0 + 1],
                                     op.mult, op.add)
        nc.sync.dma_start(o1[:, 0 : h0 + 1 - K], t_X1[:, K : h0 + 1])
        nc.vector.scalar_tensor_tensor(t_c[:, l1:h1], t_X0[:, l1:h1], cst(12),
                                       t_SB[:, l1:h1], op.mult, op.add)
        nc.scalar.dma_start(o0[:, h0 + 1 - K : L], t_X0[:, h0 + 1 : W])
        nc.vector.tensor_tensor_scan(t_X1[:, l1 + 1 : h1 + 1], t_SA[:, l1:h1],
                                     t_c[:, l1:h1], t_X1[:, l1 : l1 + 1],
                                     op.mult, op.add)
        nc.sync.dma_start(o1[:, h0 + 1 - K : L], t_X1[:, h0 + 1 : W])

        # lane-0 strip for core 0 (t in [K, L))
        nc.sync.dma_start(oz[0:1, :], t_X0[0:1, K:L])
        nc.scalar.dma_start(oz[1:2, :], t_X1[0:1, K:L])

    nc.compile()
    _cache["nc"] = nc
    return nc


def _derive(params, x0):
    M, Cc, UA2, Cp, lam, lams, F1, X1p, F3, T1, T200 = [float(params[i]) for i in range(11)]
    UA1 = H * (F1 + F3)
    k1 = (UA1 + F1 * Cp) / lam
    p_ = k1 * B
    q_ = k1 * A
    alpha_u = UA1 * F_ / lam
    alpha_c = (UA1 * G + F1 * Cp * T1) / lam - k1 * C_
    c01 = F1 * X1p / M
    c02 = p_ / M
    c03 = q_ / M
    a10 = -p_ / Cc
    i0, i1 = float(x0[0]), float(x0[1])

    cv = np.zeros(NC_CONST, np.float64)
    cv[0] = 2.0 * Cp
    cv[1] = UA2
    cv[2] = 2.0 * Cp * UA2
    cv[3] = -D / (lam * Cc)               # cA2
    cv[4] = 1.0 - q_ / Cc                 # cA1
    cv[5] = alpha_u / Cc                  # cB2
    cv[6] = alpha_c / Cc                  # cB1
    cv[7] = -(E - T200) / (lam * Cc)      # cB3
    cv[8] = alpha_u / M                   # cC2
    cv[9] = 1.0 - (F1 - alpha_c) / M      # cC1
    cv[10] = c02 * i0 + c03 * i1          # sweep-1 a offset
    cv[11] = c01                          # scan0 additive const
    cv[12] = a10                          # c coefficient
    cv[13] = -c02
    cv[14] = -c03
    cv[15] = i0
    cv[16] = i1
    return cv.astype(np.float32)


def _make_in_maps(u, x0, params):
    u = np.ascontiguousarray(u, np.float32)
    cons = np.tile(_derive(params, x0)[None, :], (P, 1))
    in_maps = []
    for c in range(NCORES):
        if c == 0:
            slab = np.concatenate([np.repeat(u[0:1], K, axis=0), u[0:TC]], axis=0)
            zr = u[0:W].reshape(1, 2 * W)
        else:
            slab = u[c * TC - K : c * TC + TC]
            zr = slab[0:W].reshape(1, 2 * W)
        in_maps.append({
            "uslab": np.ascontiguousarray(slab),
            "zrow": np.ascontiguousarray(zr),
            "cons": cons,
        })
    return in_maps


def _host_head(u, x0, params, n):
    # exact fp32 simulation of the first n steps (segment 0 has no spin-up)
    f = np.float32
    M, Cc, UA2, Cp, lam, lams, F1, X1p, F3, T1, T200 = [f(params[i]) for i in range(11)]
    out = np.empty((n, 2), f)
    s0, s1 = f(x0[0]), f(x0[1])
    fA, fB, fC, fD, fE, fF, fG, fH = f(A), f(B), f(C_), f(D), f(E), f(F_), f(G), f(H)
    one, two = f(1.0), f(2.0)
    UA1 = fH * (F1 + F3)
    for t in range(n):
        out[t, 0] = s0
        out[t, 1] = s1
        u0, u1 = f(u[t, 0]), f(u[t, 1])
        T2 = fA * s1 + fB * s0 + fC
        T3 = fD * s1 + fE
        T100 = fF * u0 + fG
        Q100 = UA1 * (T100 - T2)
        Q200 = UA2 * (T3 - T200) / (one + UA2 / (two * Cp * u1))
        F5 = Q200 / lam
        F4 = (Q100 - F1 * Cp * (T2 - T1)) / lam
        F2 = F1 - F4
        X2d = (F1 * X1p - F2 * s0) / M
        P2d = (F4 - F5) / Cc
        s0 = s0 + X2d
        s1 = s1 + P2d
    return out


def _assemble(results, head):
    out = np.empty((T, 2), np.float32)
    for c in range(NCORES):
        out[c * TC : (c + 1) * TC, 0] = results[c]["o0"].reshape(-1)
        out[c * TC : (c + 1) * TC, 1] = results[c]["o1"].reshape(-1)
    out[K:L, 0] = results[0]["oz"][0]
    out[K:L, 1] = results[0]["oz"][1]
    out[0:K] = head
    return out


def run(u_forced, x0, params, trace=False):
    from concourse.bass_utils import run_bass_kernel_spmd
    nc = _build_nc()
    in_maps = _make_in_maps(u_forced, x0, params)
    head = _host_head(u_forced, x0, params, K)
    res = run_bass_kernel_spmd(nc, in_maps, list(range(NCORES)), trace=trace)
    return _assemble(res.results, head), res


def kernel(u_forced, x0, params):
    out, _ = run(u_forced, x0, params, trace=False)
    return out
